# revision 1
# baseline (speedup 1.0000x reference)
import numpy as np
import jax
import jax.numpy as jnp
from functools import partial

# Problem constants (nn_GaussianEmbedding): hardcoded per harness contract.
NUM_TERMS = 8      # h has NUM_TERMS+1 = 9 rows
NUM_CHANNELS = 64
BATCH = 8
NODES = 2048


def _diags_one(A):
    # A: [N, N] -> D: [T, N] where D[i] = diag(A^(i+1))
    diags = []
    Ap = A
    for i in range(NUM_TERMS):
        if i > 0:
            Ap = jnp.matmul(Ap, A)
        diags.append(jnp.diagonal(Ap))
    return jnp.stack(diags, axis=0)  # [T, N]


def _per_core(A_b, h):
    # A_b: [N, N] (one batch element per core), h: [T+1, C]
    D = _diags_one(A_b)                       # [T, N]
    out = jnp.einsum("tn,tc->nc", D, h[1:])   # [N, C]
    return out + h[0][None, :]


_pmapped = None


def _get_pmapped():
    global _pmapped
    if _pmapped is None:
        _pmapped = jax.pmap(_per_core, in_axes=(0, None))
    return _pmapped


def kernel(A: np.ndarray, h: np.ndarray) -> np.ndarray:
    A = np.asarray(A, dtype=np.float32)
    h = np.asarray(h, dtype=np.float32)
    n_dev = jax.local_device_count()
    if n_dev >= BATCH:
        # Data-parallel over batch: one 2048x2048 chain of 7 matmuls per core.
        out = _get_pmapped()(A, h)            # [B, N, C]
        return np.asarray(out, dtype=np.float32)
    # Fallback: single-device vmap (correctness path).
    out = jax.vmap(_per_core, in_axes=(0, None))(jnp.asarray(A), jnp.asarray(h))
    return np.asarray(out, dtype=np.float32)

